# revision 5
# baseline (speedup 1.0000x reference)
"""GraphSAGE (4-layer) forward pass on 8 Trainium2 NeuronCores.

Strategy (dst-partitioned GNN message passing):
  - Nodes are sharded contiguously across the 8 cores (graph-aligned so the
    final per-graph pooling never crosses cores).
  - Each layer: project local shard h @ wn -> bf16 "table" shard, AllGather
    table shards into every core's DRAM, then each core processes the edges
    whose dst lies in its shard: dma_gather rows table[src] (256B rows),
    segment-sum via TensorE matmuls against a one-hot-scaled selector S built
    on VectorE (S[k,m] = (dst_k==m) * 1/deg), accumulated in PSUM per
    128-node block, fused with the self path (ws) and bias+ReLU on ScalarE.
  - Layer 4 pools directly per graph (selector keyed by graph id), so the
    output is [graphs, 5] without materializing h4.

The host side only does sharding/packing: edge partitioning by (dst block,
src table-quarter), int16 gather indices, per-edge dst/graph/1-deg metadata
columns, and degree counts (graph structure preprocessing).
"""

import os
import sys
from dataclasses import dataclass, field

import numpy as np

for _p in ("/opt/trn_rl_repo", "/root/.axon_site/_ro/trn_rl_repo"):
    if os.path.isdir(_p) and _p not in sys.path:
        sys.path.append(_p)

import ml_dtypes

BF16 = ml_dtypes.bfloat16


# --------------------------------------------------------------------------
# configuration
# --------------------------------------------------------------------------
@dataclass
class Cfg:
    gn: int  # nodes per graph
    gpc: list  # graphs per core (len 8)
    np_pad: int  # padded nodes per core (multiple of 128)
    dims: list  # [d0, d1, d2, d3, d4]
    grp: int = 8  # dst blocks per psum group

    ncores: int = 8

    @property
    def nb(self):  # 128-node blocks per core
        return self.np_pad // 128

    @property
    def table_rows(self):
        return self.ncores * self.np_pad

    @property
    def q4(self):  # rows per table quarter
        assert self.table_rows % 4 == 0
        q = self.table_rows // 4
        assert q <= 32767, q
        return q

    @property
    def g13(self):  # max graphs per core (padded graph count)
        return max(self.gpc)

    @property
    def node_lo(self):
        lo = [0]
        for c in range(self.ncores):
            lo.append(lo[-1] + self.gpc[c] * self.gn)
        return lo

    @property
    def ngrp(self):
        return (self.nb + self.grp - 1) // self.grp


FULL_CFG = Cfg(
    gn=1000,
    gpc=[13, 13, 13, 13, 12, 12, 12, 12],
    np_pad=13056,
    dims=[128, 128, 118, 103, 5],
    grp=8,
)


# --------------------------------------------------------------------------
# host-side preprocessing (sharding / packing of the graph structure)
# --------------------------------------------------------------------------
def preprocess(cfg: Cfg, src: np.ndarray, dst: np.ndarray):
    """Pack per-core edge streams.

    Returns dict with per-core arrays + the uniform tile count T.
    Edge stream layout per core: for group gi, for quarter q, for block b in
    group: a run of capacity T*128 edge slots (pad slots idx=-1).
    """
    n = cfg.node_lo[-1]
    src = np.asarray(src).astype(np.int64)
    dst = np.asarray(dst).astype(np.int64)
    deg = np.bincount(dst, minlength=n).astype(np.float64)
    invdeg = 1.0 / np.clip(deg, 1.0, None)

    # global node -> table row
    lo = np.asarray(cfg.node_lo[:-1])
    core_of = np.searchsorted(np.asarray(cfg.node_lo[1:]), np.arange(n), side="right")
    trow_of = core_of * cfg.np_pad + (np.arange(n) - lo[core_of])

    per_core = []
    max_run = 1
    for c in range(cfg.ncores):
        m = (dst >= cfg.node_lo[c]) & (dst < cfg.node_lo[c + 1])
        es, ed = src[m], dst[m]
        ld = ed - cfg.node_lo[c]
        b = ld // 128
        tr = trow_of[es]
        q = tr // cfg.q4
        lidx = tr - q * cfg.q4
        gi = b // cfg.grp
        order = np.lexsort((ld, b, q, gi))
        es, ed, ld, b, tr, q, lidx, gi = (
            x[order] for x in (es, ed, ld, b, tr, q, lidx, gi)
        )
        # run id = (gi, q, b) in processing order
        key = (gi * 4 + q) * cfg.nb + b
        per_core.append(dict(key=key, ld=ld, b=b, q=q, lidx=lidx, gi=gi, ed=ed))
        if len(key):
            _, cnts = np.unique(key, return_counts=True)
            max_run = max(max_run, int(cnts.max()))

    T = (max_run + 127) // 128
    sl = cfg.nb * 4 * T * 128  # stream length per core
    ncalls = cfg.ngrp * 4

    out = dict(T=T, sl=sl, ncalls=ncalls, cores=[])
    for c in range(cfg.ncores):
        d = per_core[c]
        key, ld, q, lidx, b = d["key"], d["ld"], d["q"], d["lidx"], d["b"]
        gi = d["gi"]
        # rank within run
        if len(key):
            kchange = np.r_[True, key[1:] != key[:-1]]
            run_start = np.maximum.accumulate(np.where(kchange, np.arange(len(key)), 0))
            rank = np.arange(len(key)) - run_start
        else:
            rank = np.zeros(0, np.int64)
        # base position of each run in the stream:
        # runs are laid out in (gi, q, b) order with capacity T*128 each,
        # groups sized by their block count.
        grp_sizes = [
            min(cfg.grp, cfg.nb - g * cfg.grp) for g in range(cfg.ngrp)
        ]
        grp_base = np.zeros(cfg.ngrp, np.int64)
        acc = 0
        for g in range(cfg.ngrp):
            grp_base[g] = acc
            acc += grp_sizes[g] * 4 * T * 128
        assert acc == sl
        ib = b - gi * cfg.grp  # block index within group
        gsz = np.asarray(grp_sizes)[gi]
        pos = grp_base[gi] + (q * gsz + ib) * T * 128 + rank
        assert pos.max(initial=0) < sl

        idx16 = np.zeros((16, sl // 16), np.int16)
        idx16[pos % 16, pos // 16] = lidx.astype(np.int16)
        idx128 = np.tile(idx16, (8, 1))

        sdst = np.full((128, sl // 128), -1.0, np.float32)
        sdst[pos % 128, pos // 128] = (ld - b * 128).astype(np.float32)
        ivd = np.zeros((128, sl // 128), np.float32)
        ivd[pos % 128, pos // 128] = invdeg[d["ed"]]
        gdst = np.full((128, sl // 128), -1.0, np.float32)
        gdst[pos % 128, pos // 128] = (ld // cfg.gn).astype(np.float32)

        # per (gi, q) call counts
        counts = np.zeros(ncalls, np.int32)
        flat_call = gi * 4 + q
        if len(flat_call):
            u, cc = np.unique(flat_call, return_counts=True)
            counts[u] = cc
        out["cores"].append(
            dict(
                idx=idx128,
                sdst=sdst,
                ivd=ivd,
                gdst=gdst,
                counts=counts.reshape(1, -1),
            )
        )
    return out


def pack_weights(cfg: Cfg, inp: dict):
    """Pad weights/biases into fixed device layouts (shared by all cores)."""
    d = cfg.dims
    w = {}
    for l in range(1, 4):
        din, dout = d[l - 1], d[l]
        wn = np.zeros((128, 128), np.float32)
        ws = np.zeros((128, 128), np.float32)
        wn[:din, :dout] = np.asarray(inp[f"wn{l}"], np.float32)
        ws[:din, :dout] = np.asarray(inp[f"ws{l}"], np.float32)
        bb = np.zeros((128, 1), np.float32)
        bb[:dout, 0] = np.asarray(inp[f"b{l}"], np.float32)
        w[f"wn{l}"] = wn.astype(BF16)
        w[f"ws{l}"] = ws.astype(BF16)
        w[f"b{l}"] = bb
    din, dout = d[3], d[4]
    wn4 = np.zeros((128, 8), np.float32)
    ws4 = np.zeros((128, 8), np.float32)
    wn4[:din, :dout] = np.asarray(inp["wn4"], np.float32)
    ws4[:din, :dout] = np.asarray(inp["ws4"], np.float32)
    b4r = np.zeros((1, 8), np.float32)
    b4r[0, :dout] = np.asarray(inp["b4"], np.float32) * float(cfg.gn)
    w["wn4"] = wn4
    w["ws4"] = ws4
    w["b4r"] = b4r
    return w


def shard_infeat(cfg: Cfg, in_feat: np.ndarray):
    """Per-core transposed bf16 input shards [128, np_pad]."""
    d0 = cfg.dims[0]
    shards = []
    for c in range(cfg.ncores):
        lo, hi = cfg.node_lo[c], cfg.node_lo[c + 1]
        h = np.zeros((128, cfg.np_pad), np.float32)
        h[:d0, : hi - lo] = np.asarray(in_feat[lo:hi], np.float32).T
        shards.append(h.astype(BF16))
    return shards


# --------------------------------------------------------------------------
# device program
# --------------------------------------------------------------------------
def build_nc(cfg: Cfg, T: int, sl: int, ncalls: int, debug_stop: int = 0, no_collective: bool = False):
    from concourse import bacc, bass, tile, mybir

    dt = mybir.dt
    d = cfg.dims
    NB, GRP, NGRP = cfg.nb, cfg.grp, cfg.ngrp
    NP, TR, Q4, GN, G13 = cfg.np_pad, cfg.table_rows, cfg.q4, cfg.gn, cfg.g13

    NQ = int(os.environ.get("GATHER_QUEUES", "1"))
    nc = bacc.Bacc(
        "TRN2",
        target_bir_lowering=False,
        debug=False,
        num_devices=cfg.ncores,
        num_swdge_queues=NQ,
        dynamic_dma_scratch_size=int(os.environ.get("DMA_SCRATCH", "16384")),
    )

    # ---- I/O -------------------------------------------------------------
    h0t_d = nc.dram_tensor("h0t", [128, NP], dt.bfloat16, kind="ExternalInput")
    idx_d = nc.dram_tensor("idx", [128, sl // 16], dt.int16, kind="ExternalInput")
    sdst_d = nc.dram_tensor("sdst", [128, sl // 128], dt.float32, kind="ExternalInput")
    ivd_d = nc.dram_tensor("ivd", [128, sl // 128], dt.float32, kind="ExternalInput")
    gdst_d = nc.dram_tensor("gdst", [128, sl // 128], dt.float32, kind="ExternalInput")
    wd = {}
    for l in range(1, 4):
        wd[f"wn{l}"] = nc.dram_tensor(f"wn{l}", [128, 128], dt.bfloat16, kind="ExternalInput")
        wd[f"ws{l}"] = nc.dram_tensor(f"ws{l}", [128, 128], dt.bfloat16, kind="ExternalInput")
        wd[f"b{l}"] = nc.dram_tensor(f"b{l}", [128, 1], dt.float32, kind="ExternalInput")
    wd["wn4"] = nc.dram_tensor("wn4", [128, 8], dt.float32, kind="ExternalInput")
    wd["ws4"] = nc.dram_tensor("ws4", [128, 8], dt.float32, kind="ExternalInput")
    wd["b4r"] = nc.dram_tensor("b4r", [1, 8], dt.float32, kind="ExternalInput")
    out_d = nc.dram_tensor("out", [G13, 8], dt.float32, kind="ExternalOutput")

    with tile.TileContext(nc) as tc:
        with (
            tc.tile_pool(name="resident", bufs=1) as rp,
            tc.tile_pool(name="dram", bufs=1, space="DRAM") as dp,
            tc.tile_pool(name="gather", bufs=3) as gp,
            tc.tile_pool(name="spool", bufs=4) as sp,
            tc.tile_pool(name="stage", bufs=3) as stp,
            tc.tile_pool(name="psum_agg", bufs=4, space="PSUM") as pag,
            tc.tile_pool(name="psum_misc", bufs=2, space="PSUM") as pms,
            tc.tile_pool(name="hts", bufs=2) as hp,
        ):
            # ---- resident SBUF tensors ----------------------------------
            idx_s = rp.tile([128, sl // 16], dt.int16)
            sdst_s = rp.tile([128, sl // 128], dt.float32)
            ivd_s = rp.tile([128, sl // 128], dt.float32)
            gdst_s = rp.tile([128, sl // 128], dt.float32)
            ws_s = {}
            for k, dd in wd.items():
                shape = [128, 128] if k[0] == "w" and k[-1] != "4" else None
                if k in ("wn4", "ws4"):
                    t = rp.tile([128, 8], dt.float32, name=f"w_{k}")
                elif k == "b4r":
                    t = rp.tile([1, 8], dt.float32, name=f"w_{k}")
                elif k.startswith("b"):
                    t = rp.tile([128, 1], dt.float32, name=f"w_{k}")
                else:
                    t = rp.tile([128, 128], dt.bfloat16, name=f"w_{k}")
                ws_s[k] = t
                nc.sync.dma_start(out=t[:], in_=dd[:])
            nc.sync.dma_start(out=idx_s[:], in_=idx_d[:])
            nc.sync.dma_start(out=sdst_s[:], in_=sdst_d[:])
            nc.sync.dma_start(out=ivd_s[:], in_=ivd_d[:])
            nc.sync.dma_start(out=gdst_s[:], in_=gdst_d[:])

            swdge_sem = (
                nc.alloc_semaphore("swdge_dma")
                if os.environ.get("DBG_EDGE") == "prep"
                else None
            )

            # constants
            iota_i = rp.tile([128, 128], dt.int32)
            nc.gpsimd.iota(iota_i[:], pattern=[[1, 128]], base=0, channel_multiplier=0)
            iota_b = rp.tile([128, 128], dt.bfloat16)
            nc.vector.tensor_copy(iota_b[:], iota_i[:])
            pidx_i = rp.tile([128, 1], dt.int32)
            nc.gpsimd.iota(pidx_i[:], pattern=[[1, 1]], base=0, channel_multiplier=1)
            pidx_f = rp.tile([128, 1], dt.float32)
            nc.vector.tensor_copy(pidx_f[:], pidx_i[:])
            ident_b = rp.tile([128, 128], dt.bfloat16)
            nc.vector.tensor_scalar(
                ident_b[:], iota_b[:], pidx_f[:], None, mybir.AluOpType.is_equal
            )
            ones_row = rp.tile([1, G13], dt.float32)
            nc.vector.memset(ones_row[:], 1.0)

            # h tiles (transposed feature-major, bf16)
            ht = [None] * 4  # ht[l] = output of layer l (l=0 -> input)
            ht[0] = hp.tile([128, NP], dt.bfloat16, tag="ht", name="ht0")
            nc.sync.dma_start(out=ht[0][:], in_=h0t_d[:])

            # DRAM table + allgather input
            tables = [
                dp.tile([TR, 128], dt.bfloat16, name="tableA"),
                dp.tile([TR, 128], dt.bfloat16, name="tableB"),
            ]
            agins = [
                dp.tile([NP, 128], dt.bfloat16, name="aginA"),
                dp.tile([NP, 128], dt.bfloat16, name="aginB"),
            ]


            grp_sizes = [min(GRP, NB - g * GRP) for g in range(NGRP)]
            grp_edge_base = np.cumsum([0] + [gs * 4 * T * 128 for gs in grp_sizes])

            def project(l, src_ht, table_slot):
                """Build table for layer l (1..4) into agins/tables[table_slot].

                l<4: table = (h_{l-1} @ wn_l) node-major; l==4: table = h3
                node-major (transpose).
                """
                din = d[l - 1]
                agin = agins[table_slot]
                for b in range(NB):
                    pp = pms.tile([128, 128], dt.float32, tag="proj", bufs=2)
                    st = stp.tile([128, 128], dt.bfloat16, tag="stage")
                    if l < 4:
                        nc.tensor.matmul(
                            pp[:, :],
                            src_ht[:din, b * 128 : (b + 1) * 128],
                            ws_s[f"wn{l}"][:din, :],
                            start=True,
                            stop=True,
                        )
                        nc.scalar.copy(st[:, :], pp[:, :])
                        nc.sync.dma_start(
                            out=agin[b * 128 : (b + 1) * 128, :],
                            in_=st[:, :],
                        )
                    else:
                        ppb = pms.tile([128, 128], dt.bfloat16, tag="projT", bufs=1)
                        nc.tensor.transpose(
                            ppb[:, :din],
                            src_ht[:din, b * 128 : (b + 1) * 128],
                            ident_b[:din, :din],
                        )
                        nc.scalar.copy(st[:, :din], ppb[:, :din])
                        nc.sync.dma_start(
                            out=agin[b * 128 : (b + 1) * 128, :din],
                            in_=st[:, :din],
                        )
                if no_collective:
                    # perf-model stand-in: same bytes written to the table
                    # as the AllGather would deliver, via local DMA.
                    for r in range(cfg.ncores):
                        nc.sync.dma_start(
                            out=tables[table_slot][r * NP : (r + 1) * NP, :],
                            in_=agin[:, :],
                        )
                else:
                    nc.gpsimd.collective_compute(
                        "AllGather",
                        mybir.AluOpType.bypass,
                        replica_groups=[list(range(cfg.ncores))],
                        ins=[agin.opt()],
                        outs=[tables[table_slot].opt()],
                    )
                return tables[table_slot]

            def edge_phase(l, table, src_ht):
                """Process edges of layer l (1..4)."""
                _em = os.environ.get("DBG_EDGE", "full")
                din = d[l - 1]
                dout = d[l] if l < 4 else None
                if l == 4:
                    p4 = pms.tile([128, 16], dt.float32, tag="small4", bufs=1)
                first_mm_done = False
                call_idx = 0
                for gi in range(NGRP):
                    gsz = grp_sizes[gi]
                    banks = None
                    if l < 4:
                        nbank = (gsz + 3) // 4
                        banks = [
                            pag.tile(
                                [128, 512], dt.float32, tag="aggbank",
                                name=f"aggbank_{l}_{gi}_{jj}",
                            )
                            for jj in range(nbank)
                        ]
                    for q in range(4):
                        nt = gsz * T
                        gt = gp.tile([128, GRP * T, 128], dt.bfloat16, tag="gather")
                        e0 = int(grp_edge_base[gi]) + q * nt * 128
                        nidx = nt * 128
                        if _em in ("prep", "chunk"):
                            CH = 1024  # idx per chunk (ring-capacity sized)
                            nch = (nidx + CH - 1) // CH
                            for ci in range(nch):
                                i0 = ci * CH
                                ilen = min(CH, nidx - i0)
                                if _em == "prep":
                                    nc.gpsimd.dma_gather(
                                        gt[:, i0 // 128 : (i0 + ilen) // 128, :],
                                        table[q * Q4 : (q + 1) * Q4, :],
                                        idx_s[:, (e0 + i0) // 16 : (e0 + i0 + ilen) // 16],
                                        ilen,
                                        ilen,
                                        128,
                                        elem_step=128,
                                        single_packet=False,
                                        prepare_only=True,
                                        sem=swdge_sem,
                                    )
                                    nc.gpsimd.trigger_dma(count=None)
                                else:
                                    nc.gpsimd.dma_gather(
                                        gt[:, i0 // 128 : (i0 + ilen) // 128, :],
                                        table[q * Q4 : (q + 1) * Q4, :],
                                        idx_s[:, (e0 + i0) // 16 : (e0 + i0 + ilen) // 16],
                                        ilen,
                                        ilen,
                                        128,
                                        elem_step=128,
                                        single_packet=False,
                                        queue_num=ci % NQ,
                                    )
                        elif _em == "lingather":
                            nc.sync.dma_start(
                                out=gt[:, :nt, :],
                                in_=table[
                                    q * Q4 : q * Q4 + nidx, :
                                ].rearrange("(n p) d -> p n d", p=128),
                            )
                        elif _em != "nogather":
                            nc.gpsimd.dma_gather(
                                gt[:, :nt, :],
                                table[q * Q4 : (q + 1) * Q4, :],
                                idx_s[:, e0 // 16 : (e0 + nidx) // 16],
                                nidx,
                                nidx,
                                128,
                                elem_step=128,
                                single_packet=False,
                                queue_num=call_idx % NQ,
                            )
                        else:
                            nc.vector.memset(gt[:], 0.0)
                        for ib in range(gsz):
                            b = gi * GRP + ib
                            if l < 4:
                                ptile = banks[ib // 4]
                                pslice = ptile[:, (ib % 4) * 128 : (ib % 4 + 1) * 128]
                                # psum accumulation groups are per 2KB bank:
                                # start on the first MM touching the bank,
                                # stop on the last.
                                bank_last_ib = min(ib // 4 * 4 + 3, gsz - 1)
                                if q == 0:
                                    # self path: hsT = ws^T . h
                                    nc.tensor.matmul(
                                        pslice[:, :],
                                        ws_s[f"ws{l}"][:din, :],
                                        src_ht[:din, b * 128 : (b + 1) * 128],
                                        start=(ib % 4 == 0),
                                        stop=False,
                                    )
                            for t in range(T):
                                col = e0 // 128 + ib * T + t
                                s_n = 128 if l < 4 else 16
                                stile = sp.tile([128, 128], dt.bfloat16, tag="S")
                                key_col = (sdst_s if l < 4 else gdst_s)[
                                    :, col : col + 1
                                ]
                                if _em in ("full", "nogather", "nomm"):
                                    nc.vector.tensor_scalar(
                                        stile[:, :s_n],
                                        iota_b[:, :s_n],
                                        key_col,
                                        ivd_s[:, col : col + 1],
                                        mybir.AluOpType.is_equal,
                                        mybir.AluOpType.mult,
                                    )
                                else:
                                    nc.vector.memset(stile[:, :s_n], 0.0)
                                lhs = gt[:, ib * T + t, :] if _em != "nomm" else iota_b[:, :]
                                if l < 4:
                                    last = (
                                        (q == 3)
                                        and (t == T - 1)
                                        and (ib == bank_last_ib)
                                    )
                                    nc.tensor.matmul(
                                        pslice[:, :],
                                        lhs,
                                        stile[:, :128],
                                        start=False,
                                        stop=last,
                                    )
                                else:
                                    nc.tensor.matmul(
                                        p4[:, :],
                                        lhs,
                                        stile[:, :16],
                                        start=not first_mm_done,
                                        stop=(gi == NGRP - 1)
                                        and (q == 3)
                                        and (ib == gsz - 1)
                                        and (t == T - 1),
                                    )
                                    first_mm_done = True
                        call_idx += 1
                    # epilogue per bank (l<4): h_l = relu(psum + b)
                    if l < 4:
                        for j, ptile in enumerate(banks):
                            w = min(512, (gsz - j * 4) * 128)
                            c0 = (gi * GRP + j * 4) * 128
                            nc.scalar.activation(
                                ht[l][:dout, c0 : c0 + w],
                                ptile[:dout, :w],
                                mybir.ActivationFunctionType.Relu,
                                bias=ws_s[f"b{l}"][:dout, 0:1],
                            )
                if l == 4:
                    return p4
                return None

            # ---------------- main schedule ------------------------------
            def dbg_out(srctile):
                dbg = rp.tile([G13, 8], dt.float32, name="dbgout")
                nc.vector.tensor_copy(dbg[:, :], srctile[:G13, :8])
                nc.sync.dma_start(out=out_d[:, :], in_=dbg[:, :])

            def _sched():
              table1 = project(1, ht[0], 0)
              if debug_stop == 1:
                gdbg = rp.tile([128, 1, 128], dt.bfloat16, name="gdbg")
                nc.gpsimd.dma_gather(
                    gdbg[:, :, :], table1[0:Q4, :], idx_s[:, 0:8],
                    128, 128, 128, elem_step=128, single_packet=False,
                )
                dbg_out(gdbg[:, 0, :])
                return
              ht[1] = hp.tile([128, NP], dt.bfloat16, tag="ht", name="ht1")
              edge_phase(1, table1, ht[0])
              if debug_stop == 2:
                dbg_out(ht[1])
                return

              table2 = project(2, ht[1], 1)
              ht[2] = hp.tile([128, NP], dt.bfloat16, tag="ht", name="ht2")
              edge_phase(2, table2, ht[1])

              table3 = project(3, ht[2], 0)
              ht[3] = hp.tile([128, NP], dt.bfloat16, tag="ht", name="ht3")
              edge_phase(3, table3, ht[2])
              if debug_stop == 3:
                dbg_out(ht[3])
                return

              table4 = project(4, ht[3], 1)
              p4 = edge_phase(4, table4, ht[3])

              # pooled_h3T[f, g] = sum over graph g's node columns of h3T
              d3 = d[3]
              ph3 = rp.tile([128, G13], dt.float32)
              for g in range(G13):
                nc.vector.tensor_reduce(
                    ph3[:d3, g : g + 1],
                    ht[3][:d3, g * GN : (g + 1) * GN],
                    mybir.AxisListType.X,
                    mybir.AluOpType.add,
                )
              pagg = rp.tile([128, G13], dt.float32)
              nc.vector.tensor_copy(pagg[:d3, :], p4[:d3, :G13])

              pf = pms.tile([G13, 8], dt.float32, tag="small4", bufs=1)
              nc.tensor.matmul(
                pf[:, : d[4]], ph3[:d3, :G13], ws_s["ws4"][:d3, : d[4]],
                start=True, stop=False,
              )
              nc.tensor.matmul(
                pf[:, : d[4]], pagg[:d3, :G13], ws_s["wn4"][:d3, : d[4]],
                start=False, stop=False,
              )
              nc.tensor.matmul(
                pf[:, : d[4]], ones_row[0:1, :G13], ws_s["b4r"][0:1, : d[4]],
                start=False, stop=True,
              )
              outs = rp.tile([G13, 8], dt.float32)
              nc.vector.tensor_scalar(
                outs[:, : d[4]], pf[:, : d[4]], 1.0 / GN, None,
                mybir.AluOpType.mult,
              )
              nc.sync.dma_start(out=out_d[:, : d[4]], in_=outs[:, : d[4]])

            _sched()

    nc.compile()
    return nc


# --------------------------------------------------------------------------
# driver
# --------------------------------------------------------------------------
def make_in_maps(cfg: Cfg, inputs: dict):
    prep = preprocess(cfg, inputs["src"], inputs["dst"])
    w = pack_weights(cfg, inputs)
    shards = shard_infeat(cfg, inputs["in_feat"])
    in_maps = []
    for c in range(cfg.ncores):
        pc = prep["cores"][c]
        m = dict(
            h0t=shards[c],
            idx=pc["idx"],
            sdst=pc["sdst"],
            ivd=pc["ivd"],
            gdst=pc["gdst"],
        )
        m.update(w)
        in_maps.append(m)
    return prep, in_maps


def assemble_output(cfg: Cfg, results):
    """results: list per core of dict name->np.ndarray."""
    ngraphs = sum(cfg.gpc)
    out = np.zeros((ngraphs, cfg.dims[4]), np.float32)
    g0 = 0
    for c in range(cfg.ncores):
        r = results[c]["out"]
        out[g0 : g0 + cfg.gpc[c]] = np.asarray(r, np.float32)[: cfg.gpc[c], : cfg.dims[4]]
        g0 += cfg.gpc[c]
    return out


_CACHE = {}


def kernel(**inputs) -> np.ndarray:
    cfg = FULL_CFG
    prep, in_maps = make_in_maps(cfg, inputs)
    key = ("nc", prep["T"], prep["sl"])
    if key not in _CACHE:
        _CACHE[key] = build_nc(cfg, prep["T"], prep["sl"], prep["ncalls"])
    nc = _CACHE[key]
    from concourse.bass_utils import run_bass_kernel_spmd

    res = run_bass_kernel_spmd(nc, in_maps, core_ids=list(range(cfg.ncores)))
    return assemble_output(cfg, res.results)



# revision 7
# speedup vs baseline: 1.7798x; 1.7798x over previous
"""GraphSAGE (4-layer) forward pass on 8 Trainium2 NeuronCores.

Strategy (dst-partitioned GNN message passing):
  - Nodes are sharded contiguously across the 8 cores (graph-aligned so the
    final per-graph pooling never crosses cores).
  - Each layer: project local shard h @ wn -> bf16 "table" shard, AllGather
    table shards into every core's DRAM, then each core processes the edges
    whose dst lies in its shard: dma_gather rows table[src] (256B rows),
    segment-sum via TensorE matmuls against a one-hot-scaled selector S built
    on VectorE (S[k,m] = (dst_k==m) * 1/deg), accumulated in PSUM per
    128-node block, fused with the self path (ws) and bias+ReLU on ScalarE.
  - Layer 4 pools directly per graph (selector keyed by graph id), so the
    output is [graphs, 5] without materializing h4.

The host side only does sharding/packing: edge partitioning by (dst block,
src table-quarter), int16 gather indices, per-edge dst/graph/1-deg metadata
columns, and degree counts (graph structure preprocessing).
"""

import os
import sys
from dataclasses import dataclass, field

import numpy as np

for _p in ("/opt/trn_rl_repo", "/root/.axon_site/_ro/trn_rl_repo"):
    if os.path.isdir(_p) and _p not in sys.path:
        sys.path.append(_p)

import ml_dtypes

BF16 = ml_dtypes.bfloat16


# --------------------------------------------------------------------------
# configuration
# --------------------------------------------------------------------------
@dataclass
class Cfg:
    gn: int  # nodes per graph
    gpc: list  # graphs per core (len 8)
    np_pad: int  # padded nodes per core (multiple of 128)
    dims: list  # [d0, d1, d2, d3, d4]
    grp: int = 8  # dst blocks per psum group

    ncores: int = 8

    @property
    def nb(self):  # 128-node blocks per core
        return self.np_pad // 128

    @property
    def table_rows(self):
        return self.ncores * self.np_pad

    @property
    def q4(self):  # rows per table quarter
        assert self.table_rows % 4 == 0
        q = self.table_rows // 4
        assert q <= 32767, q
        return q

    @property
    def g13(self):  # max graphs per core (padded graph count)
        return max(self.gpc)

    @property
    def node_lo(self):
        lo = [0]
        for c in range(self.ncores):
            lo.append(lo[-1] + self.gpc[c] * self.gn)
        return lo

    @property
    def ngrp(self):
        return (self.nb + self.grp - 1) // self.grp


FULL_CFG = Cfg(
    gn=1000,
    gpc=[13, 13, 13, 13, 12, 12, 12, 12],
    np_pad=13056,
    dims=[128, 128, 118, 103, 5],
    grp=8,
)


# --------------------------------------------------------------------------
# host-side preprocessing (sharding / packing of the graph structure)
# --------------------------------------------------------------------------
def preprocess(cfg: Cfg, src: np.ndarray, dst: np.ndarray):
    """Pack per-core edge streams.

    Returns dict with per-core arrays + the uniform tile count T.
    Edge stream layout per core: for group gi, for quarter q, for block b in
    group: a run of capacity T*128 edge slots (pad slots idx=-1).
    """
    n = cfg.node_lo[-1]
    src = np.asarray(src).astype(np.int64)
    dst = np.asarray(dst).astype(np.int64)
    deg = np.bincount(dst, minlength=n).astype(np.float64)
    invdeg = 1.0 / np.clip(deg, 1.0, None)

    # global node -> table row
    lo = np.asarray(cfg.node_lo[:-1])
    core_of = np.searchsorted(np.asarray(cfg.node_lo[1:]), np.arange(n), side="right")
    trow_of = core_of * cfg.np_pad + (np.arange(n) - lo[core_of])

    per_core = []
    max_run = 1
    for c in range(cfg.ncores):
        m = (dst >= cfg.node_lo[c]) & (dst < cfg.node_lo[c + 1])
        es, ed = src[m], dst[m]
        ld = ed - cfg.node_lo[c]
        b = ld // 128
        tr = trow_of[es]
        q = tr // cfg.q4
        lidx = tr - q * cfg.q4
        gi = b // cfg.grp
        order = np.lexsort((ld, b, q, gi))
        es, ed, ld, b, tr, q, lidx, gi = (
            x[order] for x in (es, ed, ld, b, tr, q, lidx, gi)
        )
        # run id = (gi, q, b) in processing order
        key = (gi * 4 + q) * cfg.nb + b
        per_core.append(dict(key=key, ld=ld, b=b, q=q, lidx=lidx, gi=gi, ed=ed))
        if len(key):
            _, cnts = np.unique(key, return_counts=True)
            max_run = max(max_run, int(cnts.max()))

    T = (max_run + 127) // 128
    sl = cfg.nb * 4 * T * 128  # stream length per core
    ncalls = cfg.ngrp * 4

    out = dict(T=T, sl=sl, ncalls=ncalls, cores=[])
    for c in range(cfg.ncores):
        d = per_core[c]
        key, ld, q, lidx, b = d["key"], d["ld"], d["q"], d["lidx"], d["b"]
        gi = d["gi"]
        # rank within run
        if len(key):
            kchange = np.r_[True, key[1:] != key[:-1]]
            run_start = np.maximum.accumulate(np.where(kchange, np.arange(len(key)), 0))
            rank = np.arange(len(key)) - run_start
        else:
            rank = np.zeros(0, np.int64)
        # base position of each run in the stream:
        # runs are laid out in (gi, q, b) order with capacity T*128 each,
        # groups sized by their block count.
        grp_sizes = [
            min(cfg.grp, cfg.nb - g * cfg.grp) for g in range(cfg.ngrp)
        ]
        grp_base = np.zeros(cfg.ngrp, np.int64)
        acc = 0
        for g in range(cfg.ngrp):
            grp_base[g] = acc
            acc += grp_sizes[g] * 4 * T * 128
        assert acc == sl
        ib = b - gi * cfg.grp  # block index within group
        gsz = np.asarray(grp_sizes)[gi]
        pos = grp_base[gi] + (q * gsz + ib) * T * 128 + rank
        assert pos.max(initial=0) < sl

        idx16 = np.zeros((16, sl // 16), np.int16)
        idx16[pos % 16, pos // 16] = lidx.astype(np.int16)
        idx128 = np.tile(idx16, (8, 1))

        sdst = np.full((128, sl // 128), -1.0, np.float32)
        sdst[pos % 128, pos // 128] = (ld - b * 128).astype(np.float32)
        ivd = np.zeros((128, sl // 128), np.float32)
        ivd[pos % 128, pos // 128] = invdeg[d["ed"]]
        gdst = np.full((128, sl // 128), -1.0, np.float32)
        gdst[pos % 128, pos // 128] = (ld // cfg.gn).astype(np.float32)

        # per (gi, q) call counts
        counts = np.zeros(ncalls, np.int32)
        flat_call = gi * 4 + q
        if len(flat_call):
            u, cc = np.unique(flat_call, return_counts=True)
            counts[u] = cc
        out["cores"].append(
            dict(
                idx=idx128,
                sdst=sdst,
                ivd=ivd,
                gdst=gdst,
                counts=counts.reshape(1, -1),
            )
        )
    return out


def pack_weights(cfg: Cfg, inp: dict):
    """Pad weights/biases into fixed device layouts (shared by all cores)."""
    d = cfg.dims
    w = {}
    for l in range(1, 4):
        din, dout = d[l - 1], d[l]
        wn = np.zeros((128, 128), np.float32)
        ws = np.zeros((128, 128), np.float32)
        wn[:din, :dout] = np.asarray(inp[f"wn{l}"], np.float32)
        ws[:din, :dout] = np.asarray(inp[f"ws{l}"], np.float32)
        bb = np.zeros((128, 1), np.float32)
        bb[:dout, 0] = np.asarray(inp[f"b{l}"], np.float32)
        w[f"wn{l}"] = wn.astype(BF16)
        w[f"ws{l}"] = ws.astype(BF16)
        w[f"b{l}"] = bb
    din, dout = d[3], d[4]
    wn4 = np.zeros((128, 8), np.float32)
    ws4 = np.zeros((128, 8), np.float32)
    wn4[:din, :dout] = np.asarray(inp["wn4"], np.float32)
    ws4[:din, :dout] = np.asarray(inp["ws4"], np.float32)
    b4r = np.zeros((1, 8), np.float32)
    b4r[0, :dout] = np.asarray(inp["b4"], np.float32) * float(cfg.gn)
    w["wn4"] = wn4
    w["ws4"] = ws4
    w["b4r"] = b4r
    return w


def shard_infeat(cfg: Cfg, in_feat: np.ndarray):
    """Per-core transposed bf16 input shards [128, np_pad]."""
    d0 = cfg.dims[0]
    shards = []
    for c in range(cfg.ncores):
        lo, hi = cfg.node_lo[c], cfg.node_lo[c + 1]
        h = np.zeros((128, cfg.np_pad), np.float32)
        h[:d0, : hi - lo] = np.asarray(in_feat[lo:hi], np.float32).T
        shards.append(h.astype(BF16))
    return shards


# --------------------------------------------------------------------------
# device program
# --------------------------------------------------------------------------
def build_nc(cfg: Cfg, T: int, sl: int, ncalls: int, debug_stop: int = 0, no_collective: bool = False):
    from concourse import bacc, bass, tile, mybir

    dt = mybir.dt
    d = cfg.dims
    NB, GRP, NGRP = cfg.nb, cfg.grp, cfg.ngrp
    NP, TR, Q4, GN, G13 = cfg.np_pad, cfg.table_rows, cfg.q4, cfg.gn, cfg.g13

    NQ = int(os.environ.get("GATHER_QUEUES", "1"))
    nc = bacc.Bacc(
        "TRN2",
        target_bir_lowering=False,
        debug=False,
        num_devices=cfg.ncores,
        num_swdge_queues=NQ,
        dynamic_dma_scratch_size=int(os.environ.get("DMA_SCRATCH", "16384")),
    )

    # ---- I/O -------------------------------------------------------------
    h0t_d = nc.dram_tensor("h0t", [128, NP], dt.bfloat16, kind="ExternalInput")
    idx_d = nc.dram_tensor("idx", [128, sl // 16], dt.int16, kind="ExternalInput")
    sdst_d = nc.dram_tensor("sdst", [128, sl // 128], dt.float32, kind="ExternalInput")
    ivd_d = nc.dram_tensor("ivd", [128, sl // 128], dt.float32, kind="ExternalInput")
    gdst_d = nc.dram_tensor("gdst", [128, sl // 128], dt.float32, kind="ExternalInput")
    wd = {}
    for l in range(1, 4):
        wd[f"wn{l}"] = nc.dram_tensor(f"wn{l}", [128, 128], dt.bfloat16, kind="ExternalInput")
        wd[f"ws{l}"] = nc.dram_tensor(f"ws{l}", [128, 128], dt.bfloat16, kind="ExternalInput")
        wd[f"b{l}"] = nc.dram_tensor(f"b{l}", [128, 1], dt.float32, kind="ExternalInput")
    wd["wn4"] = nc.dram_tensor("wn4", [128, 8], dt.float32, kind="ExternalInput")
    wd["ws4"] = nc.dram_tensor("ws4", [128, 8], dt.float32, kind="ExternalInput")
    wd["b4r"] = nc.dram_tensor("b4r", [1, 8], dt.float32, kind="ExternalInput")
    out_d = nc.dram_tensor("out", [G13, 8], dt.float32, kind="ExternalOutput")

    with tile.TileContext(nc) as tc:
        with (
            tc.tile_pool(name="resident", bufs=1) as rp,
            tc.tile_pool(name="dram", bufs=1, space="DRAM") as dp,
            tc.tile_pool(name="gather", bufs=3) as gp,
            tc.tile_pool(name="spool", bufs=4) as sp,
            tc.tile_pool(name="stage", bufs=3) as stp,
            tc.tile_pool(name="psum_agg", bufs=4, space="PSUM") as pag,
            tc.tile_pool(name="psum_misc", bufs=2, space="PSUM") as pms,
            tc.tile_pool(name="hts", bufs=2) as hp,
        ):
            # ---- resident SBUF tensors ----------------------------------
            idx_s = rp.tile([128, sl // 16], dt.int16)
            sdst_s = rp.tile([128, sl // 128], dt.float32)
            ivd_s = rp.tile([128, sl // 128], dt.float32)
            gdst_s = rp.tile([128, sl // 128], dt.float32)
            ws_s = {}
            for k, dd in wd.items():
                shape = [128, 128] if k[0] == "w" and k[-1] != "4" else None
                if k in ("wn4", "ws4"):
                    t = rp.tile([128, 8], dt.float32, name=f"w_{k}")
                elif k == "b4r":
                    t = rp.tile([1, 8], dt.float32, name=f"w_{k}")
                elif k.startswith("b"):
                    t = rp.tile([128, 1], dt.float32, name=f"w_{k}")
                else:
                    t = rp.tile([128, 128], dt.bfloat16, name=f"w_{k}")
                ws_s[k] = t
                nc.sync.dma_start(out=t[:], in_=dd[:])
            nc.sync.dma_start(out=idx_s[:], in_=idx_d[:])
            nc.sync.dma_start(out=sdst_s[:], in_=sdst_d[:])
            nc.sync.dma_start(out=ivd_s[:], in_=ivd_d[:])
            nc.sync.dma_start(out=gdst_s[:], in_=gdst_d[:])

            swdge_sem = (
                nc.alloc_semaphore("swdge_dma")
                if os.environ.get("DBG_EDGE") == "prep"
                else None
            )

            # constants
            iota_i = rp.tile([128, 128], dt.int32)
            nc.gpsimd.iota(iota_i[:], pattern=[[1, 128]], base=0, channel_multiplier=0)
            iota_b = rp.tile([128, 128], dt.bfloat16)
            nc.vector.tensor_copy(iota_b[:], iota_i[:])
            pidx_i = rp.tile([128, 1], dt.int32)
            nc.gpsimd.iota(pidx_i[:], pattern=[[1, 1]], base=0, channel_multiplier=1)
            pidx_f = rp.tile([128, 1], dt.float32)
            nc.vector.tensor_copy(pidx_f[:], pidx_i[:])
            ident_b = rp.tile([128, 128], dt.bfloat16)
            nc.vector.tensor_scalar(
                ident_b[:], iota_b[:], pidx_f[:], None, mybir.AluOpType.is_equal
            )
            ones_row = rp.tile([1, G13], dt.float32)
            nc.vector.memset(ones_row[:], 1.0)

            # h tiles (transposed feature-major, bf16)
            ht = [None] * 4  # ht[l] = output of layer l (l=0 -> input)
            ht[0] = hp.tile([128, NP], dt.bfloat16, tag="ht", name="ht0")
            nc.sync.dma_start(out=ht[0][:], in_=h0t_d[:])

            # DRAM table + allgather input
            tables = [
                dp.tile([TR, 128], dt.bfloat16, name="tableA"),
                dp.tile([TR, 128], dt.bfloat16, name="tableB"),
            ]
            agins = [
                dp.tile([NP, 128], dt.bfloat16, name="aginA"),
                dp.tile([NP, 128], dt.bfloat16, name="aginB"),
            ]


            grp_sizes = [min(GRP, NB - g * GRP) for g in range(NGRP)]
            grp_edge_base = np.cumsum([0] + [gs * 4 * T * 128 for gs in grp_sizes])

            def project(l, src_ht, table_slot):
                """Build table for layer l (1..4) into agins/tables[table_slot].

                l<4: table = (h_{l-1} @ wn_l) node-major; l==4: table = h3
                node-major (transpose).
                """
                din = d[l - 1]
                agin = agins[table_slot]
                for b in range(NB):
                    pp = pms.tile([128, 128], dt.float32, tag="proj", bufs=2)
                    st = stp.tile([128, 128], dt.bfloat16, tag="stage")
                    if l < 4:
                        nc.tensor.matmul(
                            pp[:, :],
                            src_ht[:din, b * 128 : (b + 1) * 128],
                            ws_s[f"wn{l}"][:din, :],
                            start=True,
                            stop=True,
                        )
                        nc.scalar.copy(st[:, :], pp[:, :])
                        nc.sync.dma_start(
                            out=agin[b * 128 : (b + 1) * 128, :],
                            in_=st[:, :],
                        )
                    else:
                        ppb = pms.tile([128, 128], dt.bfloat16, tag="projT", bufs=1)
                        nc.tensor.transpose(
                            ppb[:, :din],
                            src_ht[:din, b * 128 : (b + 1) * 128],
                            ident_b[:din, :din],
                        )
                        nc.scalar.copy(st[:, :din], ppb[:, :din])
                        nc.sync.dma_start(
                            out=agin[b * 128 : (b + 1) * 128, :din],
                            in_=st[:, :din],
                        )
                if no_collective:
                    # perf-model stand-in: same bytes written to the table
                    # as the AllGather would deliver, via local DMA.
                    for r in range(cfg.ncores):
                        nc.sync.dma_start(
                            out=tables[table_slot][r * NP : (r + 1) * NP, :],
                            in_=agin[:, :],
                        )
                else:
                    nc.gpsimd.collective_compute(
                        "AllGather",
                        mybir.AluOpType.bypass,
                        replica_groups=[list(range(cfg.ncores))],
                        ins=[agin.opt()],
                        outs=[tables[table_slot].opt()],
                    )
                return tables[table_slot]

            def edge_phase(l, table, src_ht):
                """Process edges of layer l (1..4)."""
                _em = os.environ.get("DBG_EDGE", "full")
                din = d[l - 1]
                dout = d[l] if l < 4 else None
                if l == 4:
                    p4 = pms.tile([128, 16], dt.float32, tag="small4", bufs=1)
                first_mm_done = False
                call_idx = 0
                for gi in range(NGRP):
                    gsz = grp_sizes[gi]
                    banks = None
                    if l < 4:
                        nbank = (gsz + 3) // 4
                        banks = [
                            pag.tile(
                                [128, 512], dt.float32, tag="aggbank",
                                name=f"aggbank_{l}_{gi}_{jj}",
                            )
                            for jj in range(nbank)
                        ]
                    for q in range(4):
                        nt = gsz * T
                        gt = gp.tile([128, GRP * T, 128], dt.bfloat16, tag="gather")
                        e0 = int(grp_edge_base[gi]) + q * nt * 128
                        nidx = nt * 128
                        if _em in ("prep", "chunk"):
                            CH = int(os.environ.get("GATHER_CHUNK", "1024"))
                            nch = (nidx + CH - 1) // CH
                            for ci in range(nch):
                                i0 = ci * CH
                                ilen = min(CH, nidx - i0)
                                if _em == "prep":
                                    nc.gpsimd.dma_gather(
                                        gt[:, i0 // 128 : (i0 + ilen) // 128, :],
                                        table[q * Q4 : (q + 1) * Q4, :],
                                        idx_s[:, (e0 + i0) // 16 : (e0 + i0 + ilen) // 16],
                                        ilen,
                                        ilen,
                                        128,
                                        elem_step=128,
                                        single_packet=os.environ.get('GATHER_SP','0')=='1',
                                        prepare_only=True,
                                        sem=swdge_sem,
                                    )
                                    nc.gpsimd.trigger_dma(count=None)
                                else:
                                    nc.gpsimd.dma_gather(
                                        gt[:, i0 // 128 : (i0 + ilen) // 128, :],
                                        table[q * Q4 : (q + 1) * Q4, :],
                                        idx_s[:, (e0 + i0) // 16 : (e0 + i0 + ilen) // 16],
                                        ilen,
                                        ilen,
                                        128,
                                        elem_step=128,
                                        single_packet=os.environ.get('GATHER_SP','0')=='1',
                                        queue_num=ci % NQ,
                                    )
                        elif _em == "lingather":
                            nc.sync.dma_start(
                                out=gt[:, :nt, :],
                                in_=table[
                                    q * Q4 : q * Q4 + nidx, :
                                ].rearrange("(n p) d -> p n d", p=128),
                            )
                        elif _em != "nogather":
                            nc.gpsimd.dma_gather(
                                gt[:, :nt, :],
                                table[q * Q4 : (q + 1) * Q4, :],
                                idx_s[:, e0 // 16 : (e0 + nidx) // 16],
                                nidx,
                                nidx,
                                128,
                                elem_step=128,
                                single_packet=os.environ.get('GATHER_SP','0')=='1',
                                queue_num=call_idx % NQ,
                            )
                        else:
                            nc.vector.memset(gt[:], 0.0)
                        for ib in range(gsz):
                            b = gi * GRP + ib
                            if l < 4:
                                ptile = banks[ib // 4]
                                pslice = ptile[:, (ib % 4) * 128 : (ib % 4 + 1) * 128]
                                # psum accumulation groups are per 2KB bank:
                                # start on the first MM touching the bank,
                                # stop on the last.
                                bank_last_ib = min(ib // 4 * 4 + 3, gsz - 1)
                                if q == 0:
                                    # self path: hsT = ws^T . h
                                    nc.tensor.matmul(
                                        pslice[:, :],
                                        ws_s[f"ws{l}"][:din, :],
                                        src_ht[:din, b * 128 : (b + 1) * 128],
                                        start=(ib % 4 == 0),
                                        stop=False,
                                    )
                            for t in range(T):
                                col = e0 // 128 + ib * T + t
                                s_n = 128 if l < 4 else 16
                                stile = sp.tile([128, 128], dt.bfloat16, tag="S")
                                key_col = (sdst_s if l < 4 else gdst_s)[
                                    :, col : col + 1
                                ]
                                if _em in ("full", "nogather", "nomm"):
                                    nc.vector.tensor_scalar(
                                        stile[:, :s_n],
                                        iota_b[:, :s_n],
                                        key_col,
                                        ivd_s[:, col : col + 1],
                                        mybir.AluOpType.is_equal,
                                        mybir.AluOpType.mult,
                                    )
                                else:
                                    nc.vector.memset(stile[:, :s_n], 0.0)
                                lhs = gt[:, ib * T + t, :] if _em != "nomm" else iota_b[:, :]
                                if l < 4:
                                    last = (
                                        (q == 3)
                                        and (t == T - 1)
                                        and (ib == bank_last_ib)
                                    )
                                    nc.tensor.matmul(
                                        pslice[:, :],
                                        lhs,
                                        stile[:, :128],
                                        start=False,
                                        stop=last,
                                    )
                                else:
                                    nc.tensor.matmul(
                                        p4[:, :],
                                        lhs,
                                        stile[:, :16],
                                        start=not first_mm_done,
                                        stop=(gi == NGRP - 1)
                                        and (q == 3)
                                        and (ib == gsz - 1)
                                        and (t == T - 1),
                                    )
                                    first_mm_done = True
                        call_idx += 1
                    # epilogue per bank (l<4): h_l = relu(psum + b)
                    if l < 4:
                        for j, ptile in enumerate(banks):
                            w = min(512, (gsz - j * 4) * 128)
                            c0 = (gi * GRP + j * 4) * 128
                            nc.scalar.activation(
                                ht[l][:dout, c0 : c0 + w],
                                ptile[:dout, :w],
                                mybir.ActivationFunctionType.Relu,
                                bias=ws_s[f"b{l}"][:dout, 0:1],
                            )
                if l == 4:
                    return p4
                return None

            # ---------------- main schedule ------------------------------
            def dbg_out(srctile):
                dbg = rp.tile([G13, 8], dt.float32, name="dbgout")
                nc.vector.tensor_copy(dbg[:, :], srctile[:G13, :8])
                nc.sync.dma_start(out=out_d[:, :], in_=dbg[:, :])

            def _sched():
              table1 = project(1, ht[0], 0)
              if debug_stop == 1:
                gdbg = rp.tile([128, 1, 128], dt.bfloat16, name="gdbg")
                nc.gpsimd.dma_gather(
                    gdbg[:, :, :], table1[0:Q4, :], idx_s[:, 0:8],
                    128, 128, 128, elem_step=128, single_packet=os.environ.get('GATHER_SP','0')=='1',
                )
                dbg_out(gdbg[:, 0, :])
                return
              ht[1] = hp.tile([128, NP], dt.bfloat16, tag="ht", name="ht1")
              edge_phase(1, table1, ht[0])
              if debug_stop == 2:
                dbg_out(ht[1])
                return

              table2 = project(2, ht[1], 1)
              ht[2] = hp.tile([128, NP], dt.bfloat16, tag="ht", name="ht2")
              edge_phase(2, table2, ht[1])

              table3 = project(3, ht[2], 0)
              ht[3] = hp.tile([128, NP], dt.bfloat16, tag="ht", name="ht3")
              edge_phase(3, table3, ht[2])
              if debug_stop == 3:
                dbg_out(ht[3])
                return

              table4 = project(4, ht[3], 1)
              p4 = edge_phase(4, table4, ht[3])

              # pooled_h3T[f, g] = sum over graph g's node columns of h3T
              d3 = d[3]
              ph3 = rp.tile([128, G13], dt.float32)
              for g in range(G13):
                nc.vector.tensor_reduce(
                    ph3[:d3, g : g + 1],
                    ht[3][:d3, g * GN : (g + 1) * GN],
                    mybir.AxisListType.X,
                    mybir.AluOpType.add,
                )
              pagg = rp.tile([128, G13], dt.float32)
              nc.vector.tensor_copy(pagg[:d3, :], p4[:d3, :G13])

              pf = pms.tile([G13, 8], dt.float32, tag="small4", bufs=1)
              nc.tensor.matmul(
                pf[:, : d[4]], ph3[:d3, :G13], ws_s["ws4"][:d3, : d[4]],
                start=True, stop=False,
              )
              nc.tensor.matmul(
                pf[:, : d[4]], pagg[:d3, :G13], ws_s["wn4"][:d3, : d[4]],
                start=False, stop=False,
              )
              nc.tensor.matmul(
                pf[:, : d[4]], ones_row[0:1, :G13], ws_s["b4r"][0:1, : d[4]],
                start=False, stop=True,
              )
              outs = rp.tile([G13, 8], dt.float32)
              nc.vector.tensor_scalar(
                outs[:, : d[4]], pf[:, : d[4]], 1.0 / GN, None,
                mybir.AluOpType.mult,
              )
              nc.sync.dma_start(out=out_d[:, : d[4]], in_=outs[:, : d[4]])

            _sched()

    nc.compile()
    return nc


# --------------------------------------------------------------------------
# driver
# --------------------------------------------------------------------------
def make_in_maps(cfg: Cfg, inputs: dict):
    prep = preprocess(cfg, inputs["src"], inputs["dst"])
    w = pack_weights(cfg, inputs)
    shards = shard_infeat(cfg, inputs["in_feat"])
    in_maps = []
    for c in range(cfg.ncores):
        pc = prep["cores"][c]
        m = dict(
            h0t=shards[c],
            idx=pc["idx"],
            sdst=pc["sdst"],
            ivd=pc["ivd"],
            gdst=pc["gdst"],
        )
        m.update(w)
        in_maps.append(m)
    return prep, in_maps


def assemble_output(cfg: Cfg, results):
    """results: list per core of dict name->np.ndarray."""
    ngraphs = sum(cfg.gpc)
    out = np.zeros((ngraphs, cfg.dims[4]), np.float32)
    g0 = 0
    for c in range(cfg.ncores):
        r = results[c]["out"]
        out[g0 : g0 + cfg.gpc[c]] = np.asarray(r, np.float32)[: cfg.gpc[c], : cfg.dims[4]]
        g0 += cfg.gpc[c]
    return out


_CACHE = {}


def kernel(**inputs) -> np.ndarray:
    cfg = FULL_CFG
    prep, in_maps = make_in_maps(cfg, inputs)
    key = ("nc", prep["T"], prep["sl"])
    if key not in _CACHE:
        _CACHE[key] = build_nc(cfg, prep["T"], prep["sl"], prep["ncalls"])
    nc = _CACHE[key]
    from concourse.bass_utils import run_bass_kernel_spmd

    res = run_bass_kernel_spmd(nc, in_maps, core_ids=list(range(cfg.ncores)))
    return assemble_output(cfg, res.results)

